# revision 5
# baseline (speedup 1.0000x reference)
"""LigandGINE (3-layer GINE message passing, N=100k nodes, E=600k edges,
H=128) on 8 Trainium2 NeuronCores.

Strategy
--------
Nodes are partitioned into 8 contiguous shards (12500 rows each, padded to
12544 = 98 windows of 128). Edges are partitioned by destination shard, so
scatter-aggregation is core-local. Each core holds a replicated full copy
of the node features x (allgathered between layers, bf16) in DRAM; per-edge
source rows are fetched with indirect DMA gathers.

Per 128-node window w (on its owning core), with t = 128-edge tiles:
  e_t   = eaT_t^T @ We_aug                  (PE, PSUM; ones-row folds be)
  m_t   = relu(xg_t + e_t)                  (DVE add + ACT relu, bf16)
  onehotT_t[t, wn] = (iota == dstloc_t)     (DVE is_equal, bf16)
  aggT[h, wn] += m_t^T @ onehotT_t          (PE accumulate, PSUM)
  hT    = aggT + xwinT                      (DVE; xwinT via transpose-DMA)
  s1T   = silu(W1^T @ hT + b1)              (PE + ACT, [f, n])
  xnew  = s1T^T @ W2 + b2                   (PE with s1T stationary -> row-
                                             major [n, h] directly; b2 added
                                             via a host-broadcast tile on DVE)

The TRN2 TPB ISA has a single sync-wait slot per instruction and this
walrus build refuses to split multi-wait sync_info. Tile emits multi-wait
instructions freely, so `_split_waits` moves waits onto single-wait NoOp
carriers after scheduling.
"""
import os
import sys

for _p in ('/opt/trn_rl_repo', os.path.expanduser('~/.axon_site/_ro/trn_rl_repo')):
    if os.path.isdir(_p) and _p not in sys.path:
        sys.path.append(_p)

import numpy as np
import ml_dtypes
from concourse import bass, mybir, tile
from concourse.bass_utils import run_bass_kernel_spmd

dt = mybir.dt
BF16 = np.dtype(ml_dtypes.bfloat16)

# problem constants (hardcoded per contract)
P = 128
H = 128
NCORES = 8
N_NODES = 100000
ROWS = N_NODES // NCORES          # 12500
WINS = (ROWS + P - 1) // P        # 98
RPAD = WINS * P                   # 12544
NPAD = NCORES * RPAD              # 100352
NUM_LAYERS = 3
EDGE_CH = 4
EC1 = EDGE_CH + 1                 # ones-row folds the be bias into the matmul
GT_MAX = 48                       # max gather-group size in 128-edge tiles
F32 = dt.float32
XD = dt.bfloat16                  # x-path storage/compute dtype


# ---------------------------------------------------------------- waitsplit
_PSEUDO = (mybir.InstEventSemaphore, mybir.InstNoOp)


def _split_waits(nc):
    ctr = [0]
    for bb in nc.main_func.blocks:
        out = []
        for inst in bb.instructions:
            si = inst.sync_info
            waits = list(si.on_wait) if si is not None and si.on_wait else []
            if not waits or isinstance(inst, _PSEUDO):
                out.append(inst)
                continue
            keep = 0 if inst.engine == mybir.EngineType.Pool else 1
            if len(waits) <= keep:
                out.append(inst)
                continue
            move, stay = waits[:len(waits) - keep], waits[len(waits) - keep:]
            dbg = getattr(inst, 'debug', None)
            for w in move:
                ctr[0] += 1
                nop = mybir.InstNoOp(name=f"I-wsplit-{ctr[0]}", ins=[], outs=[],
                                     engine=inst.engine)
                nop.sync_info = mybir.SyncInfo(on_wait=[w], on_update=[])
                if dbg is not None:
                    nop.debug = dbg
                out.append(nop)
            inst.sync_info = mybir.SyncInfo(
                on_wait=stay, on_update=list(si.on_update) if si.on_update else [])
            out.append(inst)
        bb.instructions = out


# ---------------------------------------------------------------- host plan
def _plan(edge_index, edge_attr):
    src = np.asarray(edge_index[0]).astype(np.int64)
    dst = np.asarray(edge_index[1]).astype(np.int64)
    ea = np.asarray(edge_attr, dtype=np.float32)
    core = dst // ROWS

    counts = np.zeros((NCORES, WINS), np.int64)
    percore = []
    for k in range(NCORES):
        sel = np.nonzero(core == k)[0]
        dloc = dst[sel] - k * ROWS
        order = np.argsort(dloc, kind='stable')
        sel = sel[order]
        dloc = dloc[order]
        win = dloc // P
        counts[k] = np.bincount(win, minlength=WINS)
        percore.append((sel, dloc, win))

    tpw = np.maximum(1, (counts.max(axis=0) + P - 1) // P).astype(np.int64)
    toff = np.concatenate([[0], np.cumsum(tpw)])
    TT = int(toff[-1])

    g_of = lambda s: (s // ROWS) * RPAD + (s % ROWS)

    srcix_all, dstloc_all, eaT_all = [], [], []
    for k in range(NCORES):
        sel, dloc, win = percore[k]
        starts = np.concatenate([[0], np.cumsum(counts[k])])
        rank = np.arange(len(sel)) - starts[win]
        pos = toff[win] * P + rank

        six = np.zeros(TT * P, np.int32)
        dlc = np.full(TT * P, 1000.0, np.float32)
        eap = np.zeros((TT * P, EC1), np.float32)
        eap[:, EDGE_CH] = 1.0
        six[pos] = g_of(src[sel]).astype(np.int32)
        dlc[pos] = (dloc % P).astype(np.float32)
        eap[pos, :EDGE_CH] = ea[sel]

        srcix_all.append(np.ascontiguousarray(six.reshape(TT, P).T))
        dstloc_all.append(np.ascontiguousarray(dlc.reshape(TT, P).T).astype(BF16))
        eaT_all.append(np.ascontiguousarray(eap.T).astype(BF16))

    groups = []  # (first_win, last_win_excl, tile_off, n_tiles)
    w0 = 0
    while w0 < WINS:
        w1 = w0
        nt = 0
        while w1 < WINS and nt + tpw[w1] <= GT_MAX:
            nt += int(tpw[w1])
            w1 += 1
        groups.append((w0, w1, int(toff[w0]), nt))
        w0 = w1

    return dict(tpw=[int(x) for x in tpw], toff=[int(x) for x in toff], TT=TT,
                groups=groups, srcix=srcix_all, dstloc=dstloc_all, eaT=eaT_all)


# ---------------------------------------------------------------- program
def _build(tpw, toff, TT, groups):
    nc = bass.Bass(target_bir_lowering=False, debug=False)

    x0s = nc.declare_dram_parameter("x0s", [RPAD, H], XD, isOutput=False)
    srcix = nc.declare_dram_parameter("srcix", [P, TT], dt.int32, isOutput=False)
    dstlc = nc.declare_dram_parameter("dstlc", [P, TT], XD, isOutput=False)
    eaT = nc.declare_dram_parameter("eaT", [EC1, TT * P], XD, isOutput=False)
    wea = nc.declare_dram_parameter("wea", [NUM_LAYERS * EC1, H], XD, isOutput=False)
    w1p = nc.declare_dram_parameter("w1p", [NUM_LAYERS * H, H], XD, isOutput=False)
    w2p = nc.declare_dram_parameter("w2p", [NUM_LAYERS * H, H], XD, isOutput=False)
    b1t = nc.declare_dram_parameter("b1t", [H, NUM_LAYERS], F32, isOutput=False)
    b2r = nc.declare_dram_parameter("b2r", [P, NUM_LAYERS * H], F32, isOutput=False)
    iotaf = nc.declare_dram_parameter("iotaf", [P, P], XD, isOutput=False)
    xout = nc.declare_dram_parameter("xout", [RPAD, H], F32, isOutput=True)

    with tile.TileContext(nc) as tc:
        with tc.tile_pool(name="dram", bufs=1, space="DRAM") as dpool, \
             tc.tile_pool(name="const", bufs=1) as cpool, \
             tc.tile_pool(name="gather", bufs=1) as gpool, \
             tc.tile_pool(name="work", bufs=3) as wk, \
             tc.tile_pool(name="winp", bufs=3) as wpool, \
             tc.tile_pool(name="pech", bufs=2, space="PSUM") as pech, \
             tc.tile_pool(name="pagg", bufs=2, space="PSUM") as pagg, \
             tc.tile_pool(name="pmlp", bufs=2, space="PSUM") as pmlp:

            xfull_l = [dpool.tile([NPAD, H], XD, addr_space="Shared",
                                  tag=f"xfull{l}", name=f"xfull{l}")
                       for l in range(NUM_LAYERS)]
            xsh = [dpool.tile([RPAD, H], XD, tag=f"xsh{i}", name=f"xsh{i}")
                   for i in range(2)]

            srcix_sb = cpool.tile([P, TT], dt.int32, tag="srcix", name="srcix_sb")
            nc.sync.dma_start(out=srcix_sb[:], in_=srcix[:])
            dstlc_sb = cpool.tile([P, TT], XD, tag="dstlc", name="dstlc_sb")
            nc.sync.dma_start(out=dstlc_sb[:], in_=dstlc[:])
            iota_sb = cpool.tile([P, P], XD, tag="iota", name="iota_sb")
            nc.sync.dma_start(out=iota_sb[:], in_=iotaf[:])
            b1_sb = cpool.tile([H, NUM_LAYERS], F32, tag="b1", name="b1_sb")
            nc.sync.dma_start(out=b1_sb[:], in_=b1t[:])
            b2_sb = cpool.tile([P, NUM_LAYERS * H], F32, tag="b2", name="b2_sb")
            nc.sync.dma_start(out=b2_sb[:], in_=b2r[:])
            wea_sb = [cpool.tile([EC1, H], XD, tag=f"wea_{l}", name=f"wea_{l}")
                      for l in range(NUM_LAYERS)]
            w1_sb = [cpool.tile([H, H], XD, tag=f"w1_{l}", name=f"w1_{l}")
                     for l in range(NUM_LAYERS)]
            w2_sb = [cpool.tile([H, H], XD, tag=f"w2_{l}", name=f"w2_{l}")
                     for l in range(NUM_LAYERS)]
            for l in range(NUM_LAYERS):
                nc.sync.dma_start(out=wea_sb[l][:], in_=wea[l * EC1:(l + 1) * EC1, :])
                nc.sync.dma_start(out=w1_sb[l][:], in_=w1p[l * H:(l + 1) * H, :])
                nc.sync.dma_start(out=w2_sb[l][:], in_=w2p[l * H:(l + 1) * H, :])

            nc.sync.dma_start(out=xsh[0][:], in_=x0s[:])

            NB = 2
            xg_bufs = [gpool.tile([P, GT_MAX, H], XD, tag=f"xg{i}", name=f"xg{i}")
                       for i in range(NB)]

            for l in range(NUM_LAYERS):
                sin = xsh[l % 2]
                sout = xsh[(l + 1) % 2]
                xfull = xfull_l[l]
                nc.gpsimd.collective_compute(
                    "AllGather", mybir.AluOpType.bypass,
                    replica_groups=[list(range(NCORES))],
                    ins=[sin.opt()], outs=[xfull.opt()],
                )

                gi = 0
                for (gw0, gw1, gtoff, gnt) in groups:
                    xg = xg_bufs[gi % NB]
                    gi += 1
                    for j in range(gnt):
                        nc.gpsimd.indirect_dma_start(
                            out=xg[:, j, :], out_offset=None,
                            in_=xfull[:],
                            in_offset=bass.IndirectOffsetOnAxis(
                                ap=srcix_sb[:, gtoff + j:gtoff + j + 1], axis=0),
                        )

                    for w in range(gw0, gw1):
                        t0 = toff[w] - gtoff
                        ntw = tpw[w]
                        eaw = wk.tile([EC1, ntw * P], XD, tag="eaw",
                                      name=f"eaw_{l}_{w}")
                        nc.sync.dma_start(
                            out=eaw[:],
                            in_=eaT[:, toff[w] * P:(toff[w] + ntw) * P])
                        aggT = pagg.tile([P, H], F32, tag="aggT",
                                         name=f"aggT_{l}_{w}")
                        ti = 0
                        while ti < ntw:
                            nch = min(4, ntw - ti)
                            ech = pech.tile([P, 4, H], F32, tag="ech",
                                            name=f"ech_{l}_{w}_{ti}")
                            for j in range(nch):
                                nc.tensor.matmul(
                                    out=ech[:, j, :],
                                    lhsT=eaw[:, (ti + j) * P:(ti + j + 1) * P],
                                    rhs=wea_sb[l][:],
                                    start=True, stop=True)
                            mraw = wk.tile([P, 4, H], F32, tag="mraw",
                                           name=f"mraw_{l}_{w}_{ti}")
                            nc.vector.tensor_add(
                                out=mraw[:, 0:nch, :],
                                in0=xg[:, t0 + ti:t0 + ti + nch, :],
                                in1=ech[:, 0:nch, :])
                            mt = wk.tile([P, 4, H], XD, tag="mt",
                                         name=f"mt_{l}_{w}_{ti}")
                            nc.scalar.activation(
                                out=mt[:, 0:nch, :], in_=mraw[:, 0:nch, :],
                                func=mybir.ActivationFunctionType.Relu)
                            for j in range(nch):
                                tt = toff[w] + ti + j
                                oh = wk.tile([P, P], XD, tag="oh",
                                             name=f"oh_{l}_{w}_{ti}_{j}")
                                nc.vector.tensor_tensor(
                                    out=oh[:], in0=iota_sb[:],
                                    in1=dstlc_sb[:, tt:tt + 1].to_broadcast([P, P]),
                                    op=mybir.AluOpType.is_equal)
                                nc.tensor.matmul(
                                    out=aggT[:], lhsT=mt[:, j, :], rhs=oh[:],
                                    start=(ti + j == 0), stop=(ti + j == ntw - 1))
                            ti += nch

                        # ---- window epilogue (transposed; no PE transposes)
                        xwT = wpool.tile([P, H], XD, tag="xwT", name=f"xwT_{l}_{w}")
                        nc.sync.dma_start_transpose(
                            out=xwT[:], in_=sin[w * P:(w + 1) * P, :])
                        hT = wpool.tile([P, H], XD, tag="hT", name=f"hT_{l}_{w}")
                        nc.vector.tensor_add(out=hT[:], in0=xwT[:], in1=aggT[:])
                        m1 = pmlp.tile([P, H], F32, tag="m1", name=f"m1_{l}_{w}")
                        nc.tensor.matmul(out=m1[:], lhsT=w1_sb[l][:], rhs=hT[:],
                                         start=True, stop=True)
                        s1T = wpool.tile([P, H], XD, tag="s1T", name=f"s1T_{l}_{w}")
                        nc.scalar.activation(out=s1T[:], in_=m1[:],
                                             func=mybir.ActivationFunctionType.Silu,
                                             bias=b1_sb[:, l:l + 1])
                        m2 = pmlp.tile([P, H], F32, tag="m2", name=f"m2_{l}_{w}")
                        nc.tensor.matmul(out=m2[:], lhsT=s1T[:], rhs=w2_sb[l][:],
                                         start=True, stop=True)
                        lastl = (l == NUM_LAYERS - 1)
                        xn = wpool.tile([P, H], F32 if lastl else XD, tag="xn",
                                        name=f"xn_{l}_{w}")
                        nc.vector.tensor_add(
                            out=xn[:], in0=m2[:],
                            in1=b2_sb[:, l * H:(l + 1) * H])
                        dst_t = xout if lastl else sout
                        nc.sync.dma_start(out=dst_t[w * P:(w + 1) * P, :], in_=xn[:])
    _split_waits(nc)
    return nc


# ---------------------------------------------------------------- entry
_PROG_CACHE = {}


def kernel(emb, We, be, W1, b1, W2, b2, edge_attr, z, edge_index, batch_vec):
    emb = np.asarray(emb, np.float32)
    We = np.asarray(We, np.float32)
    be = np.asarray(be, np.float32)
    W1 = np.asarray(W1, np.float32)
    b1 = np.asarray(b1, np.float32)
    W2 = np.asarray(W2, np.float32)
    b2 = np.asarray(b2, np.float32)
    z = np.asarray(z)
    batch_vec_np = np.asarray(batch_vec)

    plan = _plan(edge_index, edge_attr)
    key = (tuple(plan["tpw"]),)
    if key not in _PROG_CACHE:
        _PROG_CACHE[key] = _build(plan["tpw"], plan["toff"], plan["TT"],
                                  plan["groups"])
    nc = _PROG_CACHE[key]

    x0 = emb[z]
    wea_np = np.zeros((NUM_LAYERS * EC1, H), np.float32)
    for l in range(NUM_LAYERS):
        wea_np[l * EC1:l * EC1 + EDGE_CH] = We[l]
        wea_np[l * EC1 + EDGE_CH] = be[l]
    b2r_np = np.repeat(b2.reshape(1, NUM_LAYERS * H), P, axis=0).astype(np.float32)
    iota_np = np.broadcast_to(np.arange(P, dtype=np.float32), (P, P)).astype(BF16)

    in_maps = []
    for k in range(NCORES):
        x0s = np.zeros((RPAD, H), BF16)
        x0s[:ROWS] = x0[k * ROWS:(k + 1) * ROWS].astype(BF16)
        in_maps.append({
            "x0s": x0s,
            "srcix": plan["srcix"][k],
            "dstlc": plan["dstloc"][k],
            "eaT": plan["eaT"][k],
            "wea": wea_np.astype(BF16),
            "w1p": W1.reshape(NUM_LAYERS * H, H).astype(BF16),
            "w2p": W2.reshape(NUM_LAYERS * H, H).astype(BF16),
            "b1t": np.ascontiguousarray(b1.T),
            "b2r": b2r_np,
            "iotaf": iota_np,
        })

    res = run_bass_kernel_spmd(nc, in_maps, core_ids=list(range(NCORES)),
                               trace=bool(os.environ.get("GINE_TRACE")))
    kernel.last_exec_time_ns = res.exec_time_ns
    out = np.empty((N_NODES, H), np.float32)
    for k in range(NCORES):
        out[k * ROWS:(k + 1) * ROWS] = res.results[k]["xout"][:ROWS]
    return (out, batch_vec_np)


kernel.last_exec_time_ns = None


# revision 7
# speedup vs baseline: 1.4108x; 1.4108x over previous
"""LigandGINE (3-layer GINE message passing, N=100k nodes, E=600k edges,
H=128) on 8 Trainium2 NeuronCores.

Strategy
--------
Nodes are partitioned into 8 contiguous shards (12500 rows each, padded to
12544 = 98 windows of 128). Edges are partitioned by destination shard, so
scatter-aggregation is core-local. Each core holds a replicated full copy
of the node features x (allgathered between layers, bf16) in DRAM; per-edge
source rows are fetched with indirect DMA gathers.

Per 128-node window w (on its owning core), with t = 128-edge tiles:
  e_t   = eaT_t^T @ We_aug                  (PE, PSUM; ones-row folds be)
  m_t   = relu(xg_t + e_t)                  (DVE add + ACT relu, bf16)
  onehotT_t[t, wn] = (iota == dstloc_t)     (DVE is_equal, bf16)
  aggT[h, wn] += m_t^T @ onehotT_t          (PE accumulate, PSUM)
  hT    = aggT + xwinT                      (DVE; xwinT via transpose-DMA)
  s1T   = silu(W1^T @ hT + b1)              (PE + ACT, [f, n])
  xnew  = s1T^T @ W2 + b2                   (PE with s1T stationary -> row-
                                             major [n, h] directly; b2 added
                                             via a host-broadcast tile on DVE)

The TRN2 TPB ISA has a single sync-wait slot per instruction and this
walrus build refuses to split multi-wait sync_info. Tile emits multi-wait
instructions freely, so `_split_waits` moves waits onto single-wait NoOp
carriers after scheduling.
"""
import os
import sys

for _p in ('/opt/trn_rl_repo', os.path.expanduser('~/.axon_site/_ro/trn_rl_repo')):
    if os.path.isdir(_p) and _p not in sys.path:
        sys.path.append(_p)

import numpy as np
import ml_dtypes
from concourse import bass, mybir, tile
from concourse.bass_utils import run_bass_kernel_spmd

dt = mybir.dt
BF16 = np.dtype(ml_dtypes.bfloat16)

# problem constants (hardcoded per contract)
P = 128
H = 128
NCORES = 8
N_NODES = 100000
ROWS = N_NODES // NCORES          # 12500
WINS = (ROWS + P - 1) // P        # 98
RPAD = WINS * P                   # 12544
NPAD = NCORES * RPAD              # 100352
NUM_LAYERS = 3
EDGE_CH = 4
EC1 = EDGE_CH + 1                 # ones-row folds the be bias into the matmul
GT_MAX = 48                       # max gather-group size in 128-edge tiles
F32 = dt.float32
XD = dt.bfloat16                  # x-path storage/compute dtype


# ---------------------------------------------------------------- waitsplit
_PSEUDO = (mybir.InstEventSemaphore, mybir.InstNoOp)


def _split_waits(nc):
    ctr = [0]
    for bb in nc.main_func.blocks:
        out = []
        for inst in bb.instructions:
            si = inst.sync_info
            waits = list(si.on_wait) if si is not None and si.on_wait else []
            if not waits or isinstance(inst, _PSEUDO):
                out.append(inst)
                continue
            keep = 0 if inst.engine == mybir.EngineType.Pool else 1
            if len(waits) <= keep:
                out.append(inst)
                continue
            move, stay = waits[:len(waits) - keep], waits[len(waits) - keep:]
            dbg = getattr(inst, 'debug', None)
            for w in move:
                ctr[0] += 1
                nop = mybir.InstNoOp(name=f"I-wsplit-{ctr[0]}", ins=[], outs=[],
                                     engine=inst.engine)
                nop.sync_info = mybir.SyncInfo(on_wait=[w], on_update=[])
                if dbg is not None:
                    nop.debug = dbg
                out.append(nop)
            inst.sync_info = mybir.SyncInfo(
                on_wait=stay, on_update=list(si.on_update) if si.on_update else [])
            out.append(inst)
        bb.instructions = out


# ---------------------------------------------------------------- host plan
def _plan(edge_index, edge_attr):
    src = np.asarray(edge_index[0]).astype(np.int64)
    dst = np.asarray(edge_index[1]).astype(np.int64)
    ea = np.asarray(edge_attr, dtype=np.float32)
    core = dst // ROWS

    counts = np.zeros((NCORES, WINS), np.int64)
    percore = []
    for k in range(NCORES):
        sel = np.nonzero(core == k)[0]
        dloc = dst[sel] - k * ROWS
        order = np.argsort(dloc, kind='stable')
        sel = sel[order]
        dloc = dloc[order]
        win = dloc // P
        counts[k] = np.bincount(win, minlength=WINS)
        percore.append((sel, dloc, win))

    tpw = np.maximum(1, (counts.max(axis=0) + P - 1) // P).astype(np.int64)
    toff = np.concatenate([[0], np.cumsum(tpw)])
    TT = int(toff[-1])

    g_of = lambda s: (s // ROWS) * RPAD + (s % ROWS)

    srcix_all, dstloc_all, eaT_all = [], [], []
    for k in range(NCORES):
        sel, dloc, win = percore[k]
        starts = np.concatenate([[0], np.cumsum(counts[k])])
        rank = np.arange(len(sel)) - starts[win]
        pos = toff[win] * P + rank

        six = np.zeros(TT * P, np.int32)
        dlc = np.full(TT * P, 1000.0, np.float32)
        eap = np.zeros((TT * P, EC1), np.float32)
        eap[:, EDGE_CH] = 1.0
        six[pos] = g_of(src[sel]).astype(np.int32)
        dlc[pos] = (dloc % P).astype(np.float32)
        eap[pos, :EDGE_CH] = ea[sel]

        srcix_all.append(np.ascontiguousarray(six.reshape(TT, P).T))
        dstloc_all.append(np.ascontiguousarray(dlc.reshape(TT, P).T).astype(BF16))
        eaT_all.append(np.ascontiguousarray(eap.T).astype(BF16))

    groups = []  # (first_win, last_win_excl, tile_off, n_tiles)
    w0 = 0
    while w0 < WINS:
        w1 = w0
        nt = 0
        while w1 < WINS and nt + tpw[w1] <= GT_MAX:
            nt += int(tpw[w1])
            w1 += 1
        groups.append((w0, w1, int(toff[w0]), nt))
        w0 = w1

    return dict(tpw=[int(x) for x in tpw], toff=[int(x) for x in toff], TT=TT,
                groups=groups, srcix=srcix_all, dstloc=dstloc_all, eaT=eaT_all)


# ---------------------------------------------------------------- program
def _build(tpw, toff, TT, groups):
    nc = bass.Bass(target_bir_lowering=False, debug=False)

    x0s = nc.declare_dram_parameter("x0s", [RPAD, H], XD, isOutput=False)
    srcix = nc.declare_dram_parameter("srcix", [P, TT], dt.int32, isOutput=False)
    dstlc = nc.declare_dram_parameter("dstlc", [P, TT], XD, isOutput=False)
    eaT = nc.declare_dram_parameter("eaT", [EC1, TT * P], XD, isOutput=False)
    wea = nc.declare_dram_parameter("wea", [NUM_LAYERS * EC1, H], XD, isOutput=False)
    w1p = nc.declare_dram_parameter("w1p", [NUM_LAYERS * H, H], XD, isOutput=False)
    w2p = nc.declare_dram_parameter("w2p", [NUM_LAYERS * H, H], XD, isOutput=False)
    b1t = nc.declare_dram_parameter("b1t", [H, NUM_LAYERS], F32, isOutput=False)
    b2r = nc.declare_dram_parameter("b2r", [P, NUM_LAYERS * H], F32, isOutput=False)
    iotaf = nc.declare_dram_parameter("iotaf", [P, P], XD, isOutput=False)
    idenf = nc.declare_dram_parameter("idenf", [P, P], XD, isOutput=False)
    xout = nc.declare_dram_parameter("xout", [RPAD, H], F32, isOutput=True)

    with tile.TileContext(nc) as tc:
        with tc.tile_pool(name="dram", bufs=1, space="DRAM") as dpool, \
             tc.tile_pool(name="const", bufs=1) as cpool, \
             tc.tile_pool(name="gather", bufs=1) as gpool, \
             tc.tile_pool(name="work", bufs=3) as wk, \
             tc.tile_pool(name="winp", bufs=3) as wpool, \
             tc.tile_pool(name="pech", bufs=2, space="PSUM") as pech, \
             tc.tile_pool(name="pagg", bufs=2, space="PSUM") as pagg, \
             tc.tile_pool(name="pmlp", bufs=2, space="PSUM") as pmlp:

            xfull_l = [dpool.tile([NPAD, H], XD, addr_space="Shared",
                                  tag=f"xfull{l}", name=f"xfull{l}")
                       for l in range(NUM_LAYERS)]
            xsh = [dpool.tile([RPAD, H], XD, tag=f"xsh{i}", name=f"xsh{i}")
                   for i in range(2)]

            srcix_sb = cpool.tile([P, TT], dt.int32, tag="srcix", name="srcix_sb")
            nc.sync.dma_start(out=srcix_sb[:], in_=srcix[:])
            dstlc_sb = cpool.tile([P, TT], XD, tag="dstlc", name="dstlc_sb")
            nc.sync.dma_start(out=dstlc_sb[:], in_=dstlc[:])
            iota_sb = cpool.tile([P, P], XD, tag="iota", name="iota_sb")
            nc.sync.dma_start(out=iota_sb[:], in_=iotaf[:])
            iden_sb = cpool.tile([P, P], XD, tag="iden", name="iden_sb")
            nc.sync.dma_start(out=iden_sb[:], in_=idenf[:])
            b1_sb = cpool.tile([H, NUM_LAYERS], F32, tag="b1", name="b1_sb")
            nc.sync.dma_start(out=b1_sb[:], in_=b1t[:])
            b2_sb = cpool.tile([P, NUM_LAYERS * H], F32, tag="b2", name="b2_sb")
            nc.sync.dma_start(out=b2_sb[:], in_=b2r[:])
            wea_sb = [cpool.tile([EC1, H], XD, tag=f"wea_{l}", name=f"wea_{l}")
                      for l in range(NUM_LAYERS)]
            w1_sb = [cpool.tile([H, H], XD, tag=f"w1_{l}", name=f"w1_{l}")
                     for l in range(NUM_LAYERS)]
            w2_sb = [cpool.tile([H, H], XD, tag=f"w2_{l}", name=f"w2_{l}")
                     for l in range(NUM_LAYERS)]
            for l in range(NUM_LAYERS):
                nc.sync.dma_start(out=wea_sb[l][:], in_=wea[l * EC1:(l + 1) * EC1, :])
                nc.sync.dma_start(out=w1_sb[l][:], in_=w1p[l * H:(l + 1) * H, :])
                nc.sync.dma_start(out=w2_sb[l][:], in_=w2p[l * H:(l + 1) * H, :])

            nc.sync.dma_start(out=xsh[0][:], in_=x0s[:])

            NB = 2
            xg_bufs = [gpool.tile([P, GT_MAX, H], XD, tag=f"xg{i}", name=f"xg{i}")
                       for i in range(NB)]

            for l in range(NUM_LAYERS):
                sin = xsh[l % 2]
                sout = xsh[(l + 1) % 2]
                xfull = xfull_l[l]
                nc.gpsimd.collective_compute(
                    "AllGather", mybir.AluOpType.bypass,
                    replica_groups=[list(range(NCORES))],
                    ins=[sin.opt()], outs=[xfull.opt()],
                )

                gi = 0
                for (gw0, gw1, gtoff, gnt) in groups:
                    xg = xg_bufs[gi % NB]
                    gi += 1
                    for j in range(gnt):
                        nc.gpsimd.indirect_dma_start(
                            out=xg[:, j, :], out_offset=None,
                            in_=xfull[:],
                            in_offset=bass.IndirectOffsetOnAxis(
                                ap=srcix_sb[:, gtoff + j:gtoff + j + 1], axis=0),
                        )

                    for w in range(gw0, gw1):
                        t0 = toff[w] - gtoff
                        ntw = tpw[w]
                        eaw = wk.tile([EC1, ntw * P], XD, tag="eaw",
                                      name=f"eaw_{l}_{w}")
                        nc.sync.dma_start(
                            out=eaw[:],
                            in_=eaT[:, toff[w] * P:(toff[w] + ntw) * P])
                        aggT = pagg.tile([P, H], F32, tag="aggT",
                                         name=f"aggT_{l}_{w}")
                        ti = 0
                        while ti < ntw:
                            nch = min(4, ntw - ti)
                            ech = pech.tile([P, 4, H], F32, tag="ech",
                                            name=f"ech_{l}_{w}_{ti}")
                            for j in range(nch):
                                nc.tensor.matmul(
                                    out=ech[:, j, :],
                                    lhsT=eaw[:, (ti + j) * P:(ti + j + 1) * P],
                                    rhs=wea_sb[l][:],
                                    start=True, stop=True)
                            mraw = wk.tile([P, 4, H], F32, tag="mraw",
                                           name=f"mraw_{l}_{w}_{ti}")
                            nc.vector.tensor_add(
                                out=mraw[:, 0:nch, :],
                                in0=xg[:, t0 + ti:t0 + ti + nch, :],
                                in1=ech[:, 0:nch, :])
                            mt = wk.tile([P, 4, H], XD, tag="mt",
                                         name=f"mt_{l}_{w}_{ti}")
                            nc.scalar.activation(
                                out=mt[:, 0:nch, :], in_=mraw[:, 0:nch, :],
                                func=mybir.ActivationFunctionType.Relu)
                            for j in range(nch):
                                tt = toff[w] + ti + j
                                oh = wk.tile([P, P], XD, tag="oh",
                                             name=f"oh_{l}_{w}_{ti}_{j}")
                                nc.vector.tensor_tensor(
                                    out=oh[:], in0=iota_sb[:],
                                    in1=dstlc_sb[:, tt:tt + 1].to_broadcast([P, P]),
                                    op=mybir.AluOpType.is_equal)
                                nc.tensor.matmul(
                                    out=aggT[:], lhsT=mt[:, j, :], rhs=oh[:],
                                    start=(ti + j == 0), stop=False)
                            ti += nch

                        # ---- window epilogue: fold x_win into aggT via a
                        # PE transpose that joins the accumulation group
                        xw = wpool.tile([P, H], XD, tag="xw", name=f"xw_{l}_{w}")
                        nc.sync.dma_start(out=xw[:], in_=sin[w * P:(w + 1) * P, :])
                        nc.tensor.matmul(out=aggT[:], lhsT=xw[:], rhs=iden_sb[:],
                                         start=False, stop=True)
                        hT = wpool.tile([P, H], XD, tag="hT", name=f"hT_{l}_{w}")
                        nc.vector.tensor_copy(out=hT[:], in_=aggT[:])
                        m1 = pmlp.tile([P, H], F32, tag="m1", name=f"m1_{l}_{w}")
                        nc.tensor.matmul(out=m1[:], lhsT=w1_sb[l][:], rhs=hT[:],
                                         start=True, stop=True)
                        s1T = wpool.tile([P, H], XD, tag="s1T", name=f"s1T_{l}_{w}")
                        nc.scalar.activation(out=s1T[:], in_=m1[:],
                                             func=mybir.ActivationFunctionType.Silu,
                                             bias=b1_sb[:, l:l + 1])
                        m2 = pmlp.tile([P, H], F32, tag="m2", name=f"m2_{l}_{w}")
                        nc.tensor.matmul(out=m2[:], lhsT=s1T[:], rhs=w2_sb[l][:],
                                         start=True, stop=True)
                        lastl = (l == NUM_LAYERS - 1)
                        xn = wpool.tile([P, H], F32 if lastl else XD, tag="xn",
                                        name=f"xn_{l}_{w}")
                        nc.vector.tensor_add(
                            out=xn[:], in0=m2[:],
                            in1=b2_sb[:, l * H:(l + 1) * H])
                        dst_t = xout if lastl else sout
                        nc.sync.dma_start(out=dst_t[w * P:(w + 1) * P, :], in_=xn[:])
    _split_waits(nc)
    return nc


# ---------------------------------------------------------------- entry
_PROG_CACHE = {}


def kernel(emb, We, be, W1, b1, W2, b2, edge_attr, z, edge_index, batch_vec):
    emb = np.asarray(emb, np.float32)
    We = np.asarray(We, np.float32)
    be = np.asarray(be, np.float32)
    W1 = np.asarray(W1, np.float32)
    b1 = np.asarray(b1, np.float32)
    W2 = np.asarray(W2, np.float32)
    b2 = np.asarray(b2, np.float32)
    z = np.asarray(z)
    batch_vec_np = np.asarray(batch_vec)

    plan = _plan(edge_index, edge_attr)
    key = (tuple(plan["tpw"]),)
    if key not in _PROG_CACHE:
        _PROG_CACHE[key] = _build(plan["tpw"], plan["toff"], plan["TT"],
                                  plan["groups"])
    nc = _PROG_CACHE[key]

    x0 = emb[z]
    wea_np = np.zeros((NUM_LAYERS * EC1, H), np.float32)
    for l in range(NUM_LAYERS):
        wea_np[l * EC1:l * EC1 + EDGE_CH] = We[l]
        wea_np[l * EC1 + EDGE_CH] = be[l]
    b2r_np = np.repeat(b2.reshape(1, NUM_LAYERS * H), P, axis=0).astype(np.float32)
    iota_np = np.broadcast_to(np.arange(P, dtype=np.float32), (P, P)).astype(BF16)

    in_maps = []
    for k in range(NCORES):
        x0s = np.zeros((RPAD, H), BF16)
        x0s[:ROWS] = x0[k * ROWS:(k + 1) * ROWS].astype(BF16)
        in_maps.append({
            "x0s": x0s,
            "srcix": plan["srcix"][k],
            "dstlc": plan["dstloc"][k],
            "eaT": plan["eaT"][k],
            "wea": wea_np.astype(BF16),
            "w1p": W1.reshape(NUM_LAYERS * H, H).astype(BF16),
            "w2p": W2.reshape(NUM_LAYERS * H, H).astype(BF16),
            "b1t": np.ascontiguousarray(b1.T),
            "b2r": b2r_np,
            "iotaf": iota_np,
            "idenf": np.eye(P, dtype=np.float32).astype(BF16),
        })

    res = run_bass_kernel_spmd(nc, in_maps, core_ids=list(range(NCORES)),
                               trace=bool(os.environ.get("GINE_TRACE")))
    kernel.last_exec_time_ns = res.exec_time_ns
    out = np.empty((N_NODES, H), np.float32)
    for k in range(NCORES):
        out[k * ROWS:(k + 1) * ROWS] = res.results[k]["xout"][:ROWS]
    return (out, batch_vec_np)


kernel.last_exec_time_ns = None


# revision 10
# speedup vs baseline: 1.6689x; 1.1830x over previous
"""LigandGINE (3-layer GINE message passing, N=100k nodes, E=600k edges,
H=128) on 8 Trainium2 NeuronCores.

Strategy
--------
Nodes are partitioned into 8 contiguous shards (12500 rows each, padded to
12544 = 98 windows of 128). Edges are partitioned by destination shard, so
scatter-aggregation is core-local. Each core holds a replicated full copy
of the node features x (allgathered between layers, bf16) in DRAM; per-edge
source rows are fetched with indirect DMA gathers.

Per 128-node window w (on its owning core), with t = 128-edge tiles:
  e_t   = eaT_t^T @ We_aug                  (PE, PSUM; ones-row folds be)
  m_t   = relu(xg_t + e_t)                  (DVE add + ACT relu, bf16)
  onehotT_t[t, wn] = (iota == dstloc_t)     (DVE is_equal, bf16)
  aggT[h, wn] += m_t^T @ onehotT_t          (PE accumulate, PSUM)
  hT    = aggT + xwinT                      (DVE; xwinT via transpose-DMA)
  s1T   = silu(W1^T @ hT + b1)              (PE + ACT, [f, n])
  xnew  = s1T^T @ W2 + b2                   (PE with s1T stationary -> row-
                                             major [n, h] directly; b2 added
                                             via a host-broadcast tile on DVE)

The TRN2 TPB ISA has a single sync-wait slot per instruction and this
walrus build refuses to split multi-wait sync_info. Tile emits multi-wait
instructions freely, so `_split_waits` moves waits onto single-wait NoOp
carriers after scheduling.
"""
import os
import sys

for _p in ('/opt/trn_rl_repo', os.path.expanduser('~/.axon_site/_ro/trn_rl_repo')):
    if os.path.isdir(_p) and _p not in sys.path:
        sys.path.append(_p)

import numpy as np
import ml_dtypes
from concourse import bass, mybir, tile
from concourse.bass_utils import run_bass_kernel_spmd

dt = mybir.dt
BF16 = np.dtype(ml_dtypes.bfloat16)

# problem constants (hardcoded per contract)
P = 128
H = 128
NCORES = 8
N_NODES = 100000
ROWS = N_NODES // NCORES          # 12500
WINS = (ROWS + P - 1) // P        # 98
RPAD = WINS * P                   # 12544
NPAD = NCORES * RPAD              # 100352
NUM_LAYERS = 3
EDGE_CH = 4
EC1 = EDGE_CH + 1                 # ones-row folds the be bias into the matmul
GT_MAX = 24                       # max gather-group size in 128-edge tiles
F32 = dt.float32
XD = dt.bfloat16                  # x-path storage/compute dtype


# ---------------------------------------------------------------- waitsplit
_PSEUDO = (mybir.InstEventSemaphore, mybir.InstNoOp)


def _split_waits(nc):
    ctr = [0]
    for bb in nc.main_func.blocks:
        out = []
        for inst in bb.instructions:
            si = inst.sync_info
            waits = list(si.on_wait) if si is not None and si.on_wait else []
            if not waits or isinstance(inst, _PSEUDO):
                out.append(inst)
                continue
            keep = 0 if inst.engine == mybir.EngineType.Pool else 1
            if len(waits) <= keep:
                out.append(inst)
                continue
            move, stay = waits[:len(waits) - keep], waits[len(waits) - keep:]
            dbg = getattr(inst, 'debug', None)
            for w in move:
                ctr[0] += 1
                nop = mybir.InstNoOp(name=f"I-wsplit-{ctr[0]}", ins=[], outs=[],
                                     engine=inst.engine)
                nop.sync_info = mybir.SyncInfo(on_wait=[w], on_update=[])
                if dbg is not None:
                    nop.debug = dbg
                out.append(nop)
            inst.sync_info = mybir.SyncInfo(
                on_wait=stay, on_update=list(si.on_update) if si.on_update else [])
            out.append(inst)
        bb.instructions = out


# ---------------------------------------------------------------- host plan
def _plan(edge_index, edge_attr, z):
    zv = np.asarray(z).astype(np.int64)
    src = np.asarray(edge_index[0]).astype(np.int64)
    dst = np.asarray(edge_index[1]).astype(np.int64)
    ea = np.asarray(edge_attr, dtype=np.float32)
    core = dst // ROWS

    counts = np.zeros((NCORES, WINS), np.int64)
    percore = []
    for k in range(NCORES):
        sel = np.nonzero(core == k)[0]
        dloc = dst[sel] - k * ROWS
        order = np.argsort(dloc, kind='stable')
        sel = sel[order]
        dloc = dloc[order]
        win = dloc // P
        counts[k] = np.bincount(win, minlength=WINS)
        percore.append((sel, dloc, win))

    tpw = np.maximum(1, (counts.max(axis=0) + P - 1) // P).astype(np.int64)
    toff = np.concatenate([[0], np.cumsum(tpw)])
    TT = int(toff[-1])

    g_of = lambda s: (s // ROWS) * RPAD + (s % ROWS)

    srcix_all, dstloc_all, eaT_all, zsrc_all = [], [], [], []
    for k in range(NCORES):
        sel, dloc, win = percore[k]
        starts = np.concatenate([[0], np.cumsum(counts[k])])
        rank = np.arange(len(sel)) - starts[win]
        pos = toff[win] * P + rank

        six = np.zeros(TT * P, np.int32)
        zsc = np.zeros(TT * P, np.float32)
        dlc = np.full(TT * P, 1000.0, np.float32)
        eap = np.zeros((TT * P, EC1), np.float32)
        eap[:, EDGE_CH] = 1.0
        six[pos] = g_of(src[sel]).astype(np.int32)
        zsc[pos] = zv[src[sel]].astype(np.float32)
        dlc[pos] = (dloc % P).astype(np.float32)
        eap[pos, :EDGE_CH] = ea[sel]

        srcix_all.append(np.ascontiguousarray(six.reshape(TT, P).T))
        zsrc_all.append(np.ascontiguousarray(zsc.reshape(TT, P).T).astype(BF16))
        dstloc_all.append(np.ascontiguousarray(dlc.reshape(TT, P).T).astype(BF16))
        eaT_all.append(np.ascontiguousarray(eap.T).astype(BF16))

    groups = []  # (first_win, last_win_excl, tile_off, n_tiles)
    w0 = 0
    while w0 < WINS:
        w1 = w0
        nt = 0
        while w1 < WINS and nt + tpw[w1] <= GT_MAX:
            nt += int(tpw[w1])
            w1 += 1
        groups.append((w0, w1, int(toff[w0]), nt))
        w0 = w1

    return dict(tpw=[int(x) for x in tpw], toff=[int(x) for x in toff], TT=TT,
                groups=groups, srcix=srcix_all, dstloc=dstloc_all, eaT=eaT_all,
                zsrc=zsrc_all)


# ---------------------------------------------------------------- program
def _build(tpw, toff, TT, groups):
    nc = bass.Bass(target_bir_lowering=False, debug=False)

    x0s = nc.declare_dram_parameter("x0s", [RPAD, H], XD, isOutput=False)
    srcix = nc.declare_dram_parameter("srcix", [P, TT], dt.int32, isOutput=False)
    dstlc = nc.declare_dram_parameter("dstlc", [P, TT], XD, isOutput=False)
    eaT = nc.declare_dram_parameter("eaT", [EC1, TT * P], XD, isOutput=False)
    wea = nc.declare_dram_parameter("wea", [NUM_LAYERS * EC1, H], XD, isOutput=False)
    w1p = nc.declare_dram_parameter("w1p", [NUM_LAYERS * H, H], XD, isOutput=False)
    w2p = nc.declare_dram_parameter("w2p", [NUM_LAYERS * H, H], XD, isOutput=False)
    b1t = nc.declare_dram_parameter("b1t", [H, NUM_LAYERS], F32, isOutput=False)
    b2r = nc.declare_dram_parameter("b2r", [P, NUM_LAYERS * H], F32, isOutput=False)
    iotaf = nc.declare_dram_parameter("iotaf", [P, P], XD, isOutput=False)
    idenf = nc.declare_dram_parameter("idenf", [P, P], XD, isOutput=False)
    zsrcp = nc.declare_dram_parameter("zsrcp", [P, TT], XD, isOutput=False)
    embp = nc.declare_dram_parameter("embp", [P, H], XD, isOutput=False)
    xout = nc.declare_dram_parameter("xout", [RPAD, H], F32, isOutput=True)

    with tile.TileContext(nc) as tc:
        with tc.tile_pool(name="dram", bufs=1, space="DRAM") as dpool, \
             tc.tile_pool(name="const", bufs=1) as cpool, \
             tc.tile_pool(name="gather", bufs=1) as gpool, \
             tc.tile_pool(name="work", bufs=3) as wk, \
             tc.tile_pool(name="winp", bufs=3) as wpool, \
             tc.tile_pool(name="pech", bufs=2, space="PSUM") as pech, \
             tc.tile_pool(name="pagg", bufs=2, space="PSUM") as pagg, \
             tc.tile_pool(name="pmlp", bufs=1, space="PSUM") as pmlp, \
             tc.tile_pool(name="pohz", bufs=2, space="PSUM") as pohz:

            xfull_l = [dpool.tile([NPAD, H], XD, addr_space="Shared",
                                  tag=f"xfull{l}", name=f"xfull{l}")
                       for l in range(NUM_LAYERS)]
            xsh = [dpool.tile([RPAD, H], XD, tag=f"xsh{i}", name=f"xsh{i}")
                   for i in range(2)]

            srcix_sb = cpool.tile([P, TT], dt.int32, tag="srcix", name="srcix_sb")
            nc.sync.dma_start(out=srcix_sb[:], in_=srcix[:])
            dstlc_sb = cpool.tile([P, TT], XD, tag="dstlc", name="dstlc_sb")
            nc.sync.dma_start(out=dstlc_sb[:], in_=dstlc[:])
            iota_sb = cpool.tile([P, P], XD, tag="iota", name="iota_sb")
            nc.sync.dma_start(out=iota_sb[:], in_=iotaf[:])
            iden_sb = cpool.tile([P, P], XD, tag="iden", name="iden_sb")
            nc.sync.dma_start(out=iden_sb[:], in_=idenf[:])
            zsrc_sb = cpool.tile([P, TT], XD, tag="zsrc", name="zsrc_sb")
            nc.sync.dma_start(out=zsrc_sb[:], in_=zsrcp[:])
            emb_sb = cpool.tile([P, H], XD, tag="emb", name="emb_sb")
            nc.sync.dma_start(out=emb_sb[:], in_=embp[:])
            b1_sb = cpool.tile([H, NUM_LAYERS], F32, tag="b1", name="b1_sb")
            nc.sync.dma_start(out=b1_sb[:], in_=b1t[:])
            b2_sb = cpool.tile([P, NUM_LAYERS * H], F32, tag="b2", name="b2_sb")
            nc.sync.dma_start(out=b2_sb[:], in_=b2r[:])
            wea_sb = [cpool.tile([EC1, H], XD, tag=f"wea_{l}", name=f"wea_{l}")
                      for l in range(NUM_LAYERS)]
            w1_sb = [cpool.tile([H, H], XD, tag=f"w1_{l}", name=f"w1_{l}")
                     for l in range(NUM_LAYERS)]
            w2_sb = [cpool.tile([H, H], XD, tag=f"w2_{l}", name=f"w2_{l}")
                     for l in range(NUM_LAYERS)]
            for l in range(NUM_LAYERS):
                nc.sync.dma_start(out=wea_sb[l][:], in_=wea[l * EC1:(l + 1) * EC1, :])
                nc.sync.dma_start(out=w1_sb[l][:], in_=w1p[l * H:(l + 1) * H, :])
                nc.sync.dma_start(out=w2_sb[l][:], in_=w2p[l * H:(l + 1) * H, :])

            nc.sync.dma_start(out=xsh[0][:], in_=x0s[:])

            NB = 4
            xg_bufs = [gpool.tile([P, GT_MAX, H], XD, tag=f"xg{i}", name=f"xg{i}")
                       for i in range(NB)]

            for l in range(NUM_LAYERS):
                sin = xsh[l % 2]
                sout = xsh[(l + 1) % 2]
                xfull = xfull_l[l]
                if l > 0:
                    nc.gpsimd.collective_compute(
                        "AllGather", mybir.AluOpType.bypass,
                        replica_groups=[list(range(NCORES))],
                        ins=[sin.opt()], outs=[xfull.opt()],
                    )

                gi = 0
                for (gw0, gw1, gtoff, gnt) in groups:
                    xg = xg_bufs[gi % NB]
                    gi += 1
                    if l > 0:
                        for j in range(gnt):
                            nc.gpsimd.indirect_dma_start(
                                out=xg[:, j, :], out_offset=None,
                                in_=xfull[:],
                                in_offset=bass.IndirectOffsetOnAxis(
                                    ap=srcix_sb[:, gtoff + j:gtoff + j + 1],
                                    axis=0),
                            )

                    for w in range(gw0, gw1):
                        t0 = toff[w] - gtoff
                        ntw = tpw[w]
                        eaw = wk.tile([EC1, ntw * P], XD, tag="eaw",
                                      name=f"eaw_{l}_{w}")
                        nc.sync.dma_start(
                            out=eaw[:],
                            in_=eaT[:, toff[w] * P:(toff[w] + ntw) * P])
                        aggT = pagg.tile([P, H], F32, tag="aggT",
                                         name=f"aggT_{l}_{w}")
                        for j in range(ntw):
                            tt = toff[w] + j
                            ech = pech.tile([P, H], F32, tag="ech",
                                            name=f"ech_{l}_{w}_{j}")
                            if l == 0:
                                # x0[src] = emb[z[src]]: one-hot of zsrc,
                                # transposed on PE, pulls emb rows straight
                                # into the e PSUM -- no DMA gather at layer 0.
                                ohzT = wk.tile([P, P], XD, tag="ohzT",
                                               name=f"ohzT_{l}_{w}_{j}")
                                nc.vector.tensor_tensor(
                                    out=ohzT[:], in0=iota_sb[:],
                                    in1=zsrc_sb[:, tt:tt + 1].to_broadcast([P, P]),
                                    op=mybir.AluOpType.is_equal)
                                ohz_ps = pohz.tile([P, P], F32, tag="ohz",
                                                   name=f"ohzp_{l}_{w}_{j}")
                                nc.tensor.matmul(out=ohz_ps[:], lhsT=ohzT[:],
                                                 rhs=iden_sb[:],
                                                 start=True, stop=True)
                                ohz = wk.tile([P, P], XD, tag="ohz",
                                              name=f"ohz_{l}_{w}_{j}")
                                nc.scalar.copy(out=ohz[:], in_=ohz_ps[:])
                                nc.tensor.matmul(
                                    out=ech[:], lhsT=eaw[:, j * P:(j + 1) * P],
                                    rhs=wea_sb[l][:], start=True, stop=False)
                                nc.tensor.matmul(
                                    out=ech[:], lhsT=ohz[:], rhs=emb_sb[:],
                                    start=False, stop=True)
                                mt = wk.tile([P, H], XD, tag="mt",
                                             name=f"mt_{l}_{w}_{j}")
                                nc.scalar.activation(
                                    out=mt[:], in_=ech[:],
                                    func=mybir.ActivationFunctionType.Relu)
                            else:
                                nc.tensor.matmul(
                                    out=ech[:], lhsT=eaw[:, j * P:(j + 1) * P],
                                    rhs=wea_sb[l][:], start=True, stop=True)
                                mraw = wk.tile([P, H], F32, tag="mraw",
                                               name=f"mraw_{l}_{w}_{j}")
                                nc.vector.tensor_add(
                                    out=mraw[:], in0=xg[:, t0 + j, :], in1=ech[:])
                                mt = wk.tile([P, H], XD, tag="mt",
                                             name=f"mt_{l}_{w}_{j}")
                                nc.scalar.activation(
                                    out=mt[:], in_=mraw[:],
                                    func=mybir.ActivationFunctionType.Relu)
                            oh = wk.tile([P, P], XD, tag="oh",
                                         name=f"oh_{l}_{w}_{j}")
                            nc.vector.tensor_tensor(
                                out=oh[:], in0=iota_sb[:],
                                in1=dstlc_sb[:, tt:tt + 1].to_broadcast([P, P]),
                                op=mybir.AluOpType.is_equal)
                            nc.tensor.matmul(
                                out=aggT[:], lhsT=mt[:], rhs=oh[:],
                                start=(j == 0), stop=False)

                        # ---- window epilogue: fold x_win into aggT via a
                        # PE transpose that joins the accumulation group
                        xw = wpool.tile([P, H], XD, tag="xw", name=f"xw_{l}_{w}")
                        nc.sync.dma_start(out=xw[:], in_=sin[w * P:(w + 1) * P, :])
                        nc.tensor.matmul(out=aggT[:], lhsT=xw[:], rhs=iden_sb[:],
                                         start=False, stop=True)
                        hT = wpool.tile([P, H], XD, tag="hT", name=f"hT_{l}_{w}")
                        nc.vector.tensor_copy(out=hT[:], in_=aggT[:])
                        m1 = pmlp.tile([P, H], F32, tag="m1", name=f"m1_{l}_{w}")
                        nc.tensor.matmul(out=m1[:], lhsT=w1_sb[l][:], rhs=hT[:],
                                         start=True, stop=True)
                        s1T = wpool.tile([P, H], XD, tag="s1T", name=f"s1T_{l}_{w}")
                        nc.scalar.activation(out=s1T[:], in_=m1[:],
                                             func=mybir.ActivationFunctionType.Silu,
                                             bias=b1_sb[:, l:l + 1])
                        m2 = pmlp.tile([P, H], F32, tag="m2", name=f"m2_{l}_{w}")
                        nc.tensor.matmul(out=m2[:], lhsT=s1T[:], rhs=w2_sb[l][:],
                                         start=True, stop=True)
                        lastl = (l == NUM_LAYERS - 1)
                        xn = wpool.tile([P, H], F32 if lastl else XD, tag="xn",
                                        name=f"xn_{l}_{w}")
                        nc.vector.tensor_add(
                            out=xn[:], in0=m2[:],
                            in1=b2_sb[:, l * H:(l + 1) * H])
                        dst_t = xout if lastl else sout
                        nc.sync.dma_start(out=dst_t[w * P:(w + 1) * P, :], in_=xn[:])
    _split_waits(nc)
    return nc


# ---------------------------------------------------------------- entry
_PROG_CACHE = {}


def kernel(emb, We, be, W1, b1, W2, b2, edge_attr, z, edge_index, batch_vec):
    emb = np.asarray(emb, np.float32)
    We = np.asarray(We, np.float32)
    be = np.asarray(be, np.float32)
    W1 = np.asarray(W1, np.float32)
    b1 = np.asarray(b1, np.float32)
    W2 = np.asarray(W2, np.float32)
    b2 = np.asarray(b2, np.float32)
    z = np.asarray(z)
    batch_vec_np = np.asarray(batch_vec)

    plan = _plan(edge_index, edge_attr, z)
    key = (tuple(plan["tpw"]),)
    if key not in _PROG_CACHE:
        _PROG_CACHE[key] = _build(plan["tpw"], plan["toff"], plan["TT"],
                                  plan["groups"])
    nc = _PROG_CACHE[key]

    x0 = emb[z]
    emb_pad = np.zeros((P, H), np.float32)
    emb_pad[:emb.shape[0]] = emb
    emb_pad = emb_pad.astype(BF16)
    wea_np = np.zeros((NUM_LAYERS * EC1, H), np.float32)
    for l in range(NUM_LAYERS):
        wea_np[l * EC1:l * EC1 + EDGE_CH] = We[l]
        wea_np[l * EC1 + EDGE_CH] = be[l]
    b2r_np = np.repeat(b2.reshape(1, NUM_LAYERS * H), P, axis=0).astype(np.float32)
    iota_np = np.broadcast_to(np.arange(P, dtype=np.float32), (P, P)).astype(BF16)

    in_maps = []
    for k in range(NCORES):
        x0s = np.zeros((RPAD, H), BF16)
        x0s[:ROWS] = x0[k * ROWS:(k + 1) * ROWS].astype(BF16)
        in_maps.append({
            "x0s": x0s,
            "srcix": plan["srcix"][k],
            "dstlc": plan["dstloc"][k],
            "eaT": plan["eaT"][k],
            "wea": wea_np.astype(BF16),
            "w1p": W1.reshape(NUM_LAYERS * H, H).astype(BF16),
            "w2p": W2.reshape(NUM_LAYERS * H, H).astype(BF16),
            "b1t": np.ascontiguousarray(b1.T),
            "b2r": b2r_np,
            "iotaf": iota_np,
            "idenf": np.eye(P, dtype=np.float32).astype(BF16),
            "zsrcp": plan["zsrc"][k],
            "embp": emb_pad,
        })

    res = run_bass_kernel_spmd(nc, in_maps, core_ids=list(range(NCORES)),
                               trace=bool(os.environ.get("GINE_TRACE")))
    kernel.last_exec_time_ns = res.exec_time_ns
    out = np.empty((N_NODES, H), np.float32)
    for k in range(NCORES):
        out[k * ROWS:(k + 1) * ROWS] = res.results[k]["xout"][:ROWS]
    return (out, batch_vec_np)


kernel.last_exec_time_ns = None
